# revision 11
# baseline (speedup 1.0000x reference)
"""Trainium2 Bass kernel for nn_MaskPyramids (gaussian mask pyramids).

Per instance, paste a (shifted, clipped) 28x28 table crop into 5 pyramid
levels (200^2, 100^2, 50^2, 25^2, 13^2) -> out [512, 53294] fp32.
8 cores x 64 instances, data parallel, no collectives.

mask_n = R_n^T . table . C_n with 0/1 selection matrices built on-device
via is_equal compares (bit-exact gathers: each output = 1.0 * table value).

HW constraint discovered by probing: matmul tile_position must be diagonal
(sp, sp) with M <= 32, or plain (0, 0); (row strip != col strip) wedges the
device. So:
  - S1 (table gather R) runs as diagonal strip matmuls (4 instances per
    PSUM tile, one strip each),
  - level 0 (M=100) runs as (0,0) matmuls after a small SBUF->SBUF DMA
    moves each instance's gathered table GT to partition strip 0 and the
    level-0 column selections are computed directly at strip 0,
  - levels 1-4 (M <= 25, padded to 32) run as diagonal strip matmuls.
Row blocks are packed multiple-rows-per-partition so DRAM writes are
contiguous runs (1600B for levels 0-1); output DMAs are batched over
instance groups (3-dim access patterns) to keep the DMA instruction count
low.
"""

import numpy as np

import concourse.bacc as bacc
import concourse.mybir as mybir
import concourse.tile as tile
from concourse.bass import AP
from concourse.bass_utils import run_bass_kernel_spmd

F32 = mybir.dt.float32

N_CORES = 8
PER_CORE = 64
N_QUADS = 16  # 4 instances per quad
N_GRP = 4  # 4 quads per group -> 16 instances

G = 28
KP = 32
CTR = 13
INIT = 200
TOTAL = 53294

SIZES = [200, 100, 50, 25, 13]
RPP = [2, 4, 2, 1, 1]  # rows per partition
MPART = [100, 25, 25, 25, 13]  # data rows per chunk (pre-pad)
PADM = [100, 32, 32, 32, 32]  # chunk width in gt (incl. zero pad)
PW = [200, 128, 64, 32, 32]  # level width in gt = RPP * PADM
RO = [0, 200, 328, 392, 424]  # gt col offset per level
GT_W = 456
DRAM_OFF = [0, 40000, 50000, 52500, 53125]

# iota columns: R section [0, 456); C sections for levels 1-4; C level-0
C14_W = [100, 50, 25, 13]
C14_RO = [456, 556, 606, 631]
C0_RO = 644
IOTA_W = 844

# shifts columns: R (l*16+q), C0 (64 per-instance), C1-4 ((l-1)*16+q)
R_OFF, C0_OFF, C14_OFF, SH_W = 0, 80, 144, 208


def _round_f32(x, scale):
    a = x.astype(np.float32) / np.float32(INIT)
    b = a * np.float32(scale)
    return np.round(b).astype(np.int32)


def _mk(t_ap, dims, extra_off=0):
    return AP(t_ap.tensor, t_ap.offset + extra_off, dims)


def build_program():
    nc = bacc.Bacc("TRN2", target_bir_lowering=False, debug=False)

    tbl_d = nc.dram_tensor("table4", [128, KP], F32, kind="ExternalInput")
    iota_d = nc.dram_tensor("iota", [128, IOTA_W], F32, kind="ExternalInput")
    shf_d = nc.dram_tensor("shifts", [128, SH_W], F32, kind="ExternalInput")
    out_d = nc.dram_tensor("out", [PER_CORE, TOTAL], F32, kind="ExternalOutput")

    def oap(n0, lvl, dims):
        # DRAM out AP rooted at instance n0, level lvl
        return AP(out_d.ap().tensor, n0 * TOTAL + DRAM_OFF[lvl], dims)

    with tile.TileContext(nc) as tc:
        from contextlib import ExitStack

        with ExitStack() as ctx:
            consts = ctx.enter_context(tc.tile_pool(name="consts", bufs=1))
            selp = ctx.enter_context(tc.tile_pool(name="sel", bufs=1))
            c0p = ctx.enter_context(tc.tile_pool(name="c0", bufs=2))
            gtgp = ctx.enter_context(tc.tile_pool(name="gtg", bufs=2))
            gt0p = ctx.enter_context(tc.tile_pool(name="gt0", bufs=2))
            st0p = ctx.enter_context(tc.tile_pool(name="st0", bufs=2))
            stsp = ctx.enter_context(tc.tile_pool(name="stsm", bufs=2))
            psA = ctx.enter_context(tc.tile_pool(name="psA", bufs=2, space="PSUM"))
            psL0 = ctx.enter_context(tc.tile_pool(name="psL0", bufs=3, space="PSUM"))
            psSm = ctx.enter_context(tc.tile_pool(name="psSm", bufs=3, space="PSUM"))

            tbl = consts.tile([128, KP], F32, tag="tbl")
            nc.sync.dma_start(tbl[:, :], tbl_d[:, :])
            iot = consts.tile([128, IOTA_W], F32, tag="iota")
            nc.sync.dma_start(iot[:, :], iota_d[:, :])
            shf = consts.tile([128, SH_W], F32, tag="shf")
            nc.sync.dma_start(shf[:, :], shf_d[:, :])
            pstep_i = iot[:, :].ap[0]
            pstep_s = shf[:, :].ap[0]

            # ---- R selections: one tensor [128, 16*456], quad-major ----
            selR = selp.tile([128, N_QUADS * GT_W], F32, tag="selR")
            prs = selR[:, :].ap[0]
            for lv in range(5):
                w = PW[lv]
                nc.vector.tensor_tensor(
                    _mk(selR[:, :], [prs, [GT_W, N_QUADS], [1, w]], RO[lv]),
                    _mk(iot[:, :], [pstep_i, [0, N_QUADS], [1, w]], RO[lv]),
                    _mk(shf[:, :], [pstep_s, [1, N_QUADS], [0, w]], R_OFF + lv * 16),
                    mybir.AluOpType.is_equal,
                )

            # ---- C selections, levels 1-4: strip-packed [128, 16*S] ----
            selC = {}
            for i, lv in enumerate((1, 2, 3, 4)):
                w = SIZES[lv]
                t = selp.tile([128, N_QUADS * w], F32, tag=f"selC{lv}")
                nc.vector.tensor_tensor(
                    t[:, :],
                    _mk(iot[:, :], [pstep_i, [0, N_QUADS], [1, w]], C14_RO[i]),
                    _mk(shf[:, :], [pstep_s, [1, N_QUADS], [0, w]], C14_OFF + i * 16),
                    mybir.AluOpType.is_equal,
                )
                selC[lv] = t

            # ---- per group of 16 instances ----
            for grp in range(N_GRP):
                # C selections level 0 at strip 0: [32, 16*200]
                c0 = c0p.tile([32, 16 * 200], F32, tag="c0")
                nc.vector.tensor_tensor(
                    c0[:, :],
                    _mk(iot[:, :], [[pstep_i[0], 32], [0, 16], [1, 200]], C0_RO),
                    _mk(
                        shf[:, :],
                        [[pstep_s[0], 32], [1, 16], [0, 200]],
                        C0_OFF + grp * 16,
                    ),
                    mybir.AluOpType.is_equal,
                )

                # S1 + gt copy into group tile
                gtg = gtgp.tile([128, N_GRP * GT_W], F32, tag="gtg")
                for qq in range(N_GRP):
                    q = grp * N_GRP + qq
                    ps1 = psA.tile([128, 512], F32, tag="s1")
                    for g in range(4):
                        sp = 32 * g
                        nc.tensor.matmul(
                            ps1[sp : sp + KP, 0:GT_W],
                            tbl[sp : sp + KP, :],
                            selR[sp : sp + KP, q * GT_W : (q + 1) * GT_W],
                            start=True,
                            stop=True,
                            tile_position=(sp, sp),
                        )
                    nc.scalar.copy(
                        gtg[:, qq * GT_W : qq * GT_W + GT_W], ps1[:, 0:GT_W]
                    )

                # strip-move: gt level-0 sections -> strip 0 [32, 16*200]
                gt0 = gt0p.tile([32, 16 * 200], F32, tag="gt0")
                pg = gtg[:, :].ap[0]
                p0a = gt0[:, :].ap[0]
                for g in range(4):
                    sp = 32 * g
                    src = _mk(
                        gtg[sp : sp + 32, :], [[pg[0], 32], [GT_W, N_GRP], [1, 200]]
                    )
                    dst = _mk(
                        gt0[:, :], [[p0a[0], 32], [4 * 200, N_GRP], [1, 200]], g * 200
                    )
                    nc.sync.dma_start(dst, src)

                # S2 level 0: (0,0) matmuls, M=100, per instance
                for qq in range(N_GRP):
                    q = grp * N_GRP + qq
                    for g in range(4):
                        i16 = qq * 4 + g
                        n = q * 4 + g
                        p0 = psL0.tile([100, 512], F32, tag="l0")
                        for c in range(2):
                            nc.tensor.matmul(
                                p0[:, c * 200 : (c + 1) * 200],
                                gt0[0:KP, i16 * 200 + c * 100 : i16 * 200 + (c + 1) * 100],
                                c0[0:KP, i16 * 200 : (i16 + 1) * 200],
                                start=True,
                                stop=True,
                                tile_position=(0, 0),
                            )
                        if i16 % 8 == 0:
                            st0 = st0p.tile([100, 8 * 400], F32, tag="st0")
                        (nc.vector.tensor_copy if g % 2 else nc.scalar.copy)(
                            st0[:, (i16 % 8) * 400 : (i16 % 8) * 400 + 400],
                            p0[:, 0:400],
                        )
                        if i16 % 8 == 7:
                            n0 = (n // 8) * 8
                            nc.sync.dma_start(
                                oap(n0, 0, [[400, 100], [TOTAL, 8], [1, 400]]),
                                st0[:, :],
                            )

                # S2 levels 1-4: diagonal strip matmuls per quad
                lvcfg = [
                    (1, 4, 100, "st1"),  # (lvl, chunks, N, tag)
                    (2, 2, 50, "st2"),
                    (3, 1, 25, "st3"),
                    (4, 1, 13, "st4"),
                ]
                stg = {}
                for lv, nch, N, tag in lvcfg:
                    stg[lv] = stsp.tile([128, N_GRP * nch * N], F32, tag=tag, name=tag)
                for qq in range(N_GRP):
                    q = grp * N_GRP + qq
                    for lv, nch, N, tag in lvcfg:
                        F = nch * N
                        ps = psSm.tile([128, 512], F32, tag="sm")
                        for g in range(4):
                            sp = 32 * g
                            for c in range(nch):
                                nc.tensor.matmul(
                                    ps[sp : sp + 32, c * N : (c + 1) * N],
                                    gtg[
                                        sp : sp + KP,
                                        qq * GT_W + RO[lv] + c * 32 : qq * GT_W
                                        + RO[lv]
                                        + (c + 1) * 32,
                                    ],
                                    selC[lv][sp : sp + KP, q * N : (q + 1) * N],
                                    start=True,
                                    stop=True,
                                    tile_position=(sp, sp),
                                )
                        (nc.vector.tensor_copy if lv <= 2 else nc.scalar.copy)(
                            stg[lv][:, qq * F : (qq + 1) * F], ps[:, 0:F]
                        )
                # group DMAs: per strip, 4 quads x [rows, F]
                n0g = grp * 16
                for lv, nch, N, tag in lvcfg:
                    F = nch * N
                    rows = MPART[lv]
                    pstg = stg[lv][:, :].ap[0]
                    for g in range(4):
                        sp = 32 * g
                        src = _mk(
                            stg[lv][sp : sp + rows, :],
                            [[pstg[0], rows], [F, N_GRP], [1, F]],
                        )
                        dst = oap(
                            n0g + g, lv, [[F, rows], [4 * TOTAL, N_GRP], [1, F]]
                        )
                        (nc.sync if lv == 1 else nc.scalar).dma_start(dst, src)

    nc.compile()
    return nc


_PROGRAM = None


def _get_program():
    global _PROGRAM
    if _PROGRAM is None:
        _PROGRAM = build_program()
    return _PROGRAM


def host_inputs(pos, shared_mask):
    pos = np.asarray(pos)
    shared_mask = np.asarray(shared_mask, dtype=np.float32)

    table4 = np.zeros((128, KP), np.float32)
    iota = np.full((128, IOTA_W), -1e9, np.float32)
    for g in range(4):
        sp = 32 * g
        table4[sp : sp + G, :G] = shared_mask
        a = np.arange(G, dtype=np.float32)[:, None]
        for lv in range(5):
            r, M, pm = RPP[lv], MPART[lv], PADM[lv]
            colvals = np.full(PW[lv], np.nan, np.float32)
            for c in range(r):
                for m in range(M):
                    colvals[c * pm + m] = r * m + c
            blk = colvals[None, :] - a
            blk = np.where(np.isnan(blk), -1e9, blk)
            iota[sp : sp + G, RO[lv] : RO[lv] + PW[lv]] = blk
        for i, lv in enumerate((1, 2, 3, 4)):
            S = SIZES[lv]
            j = np.arange(S, dtype=np.float32)[None, :]
            iota[sp : sp + G, C14_RO[i] : C14_RO[i] + S] = j - a
        j = np.arange(200, dtype=np.float32)[None, :]
        iota[sp : sp + G, C0_RO : C0_RO + 200] = j - a

    sh = np.stack([_round_f32(pos[:, 0], S) - CTR for S in SIZES], 0)  # [5,512]
    sw = np.stack([_round_f32(pos[:, 1], S) - CTR for S in SIZES], 0)

    in_maps = []
    for c in range(N_CORES):
        shifts = np.zeros((128, SH_W), np.float32)
        for g in range(4):
            sp = 32 * g
            for lv in range(5):
                for q in range(N_QUADS):
                    n = c * PER_CORE + q * 4 + g
                    shifts[sp : sp + 32, R_OFF + lv * 16 + q] = sh[lv, n]
                    if lv >= 1:
                        shifts[sp : sp + 32, C14_OFF + (lv - 1) * 16 + q] = sw[lv, n]
        for nl in range(PER_CORE):
            shifts[:, C0_OFF + nl] = sw[0, c * PER_CORE + nl]
        in_maps.append(
            {"table4": table4, "iota": iota, "shifts": shifts.astype(np.float32)}
        )
    return in_maps


def kernel(pos, shared_mask):
    nc = _get_program()
    in_maps = host_inputs(pos, shared_mask)
    res = run_bass_kernel_spmd(nc, in_maps, list(range(N_CORES)))
    return np.concatenate([res.results[c]["out"] for c in range(N_CORES)], axis=0)


# revision 13
# speedup vs baseline: 9.7653x; 9.7653x over previous
"""Trainium2 Bass kernel for nn_MaskPyramids (gaussian mask pyramids).

Per instance, paste a (shifted, clipped) 28x28 table crop into 5 pyramid
levels (200^2, 100^2, 50^2, 25^2, 13^2) -> out [512, 53294] fp32.
8 cores x 64 instances, data parallel, no collectives.

mask_n = R_n^T . table . C_n with 0/1 selection matrices built on-device
via is_equal compares (bit-exact gathers: each output = 1.0 * table value).

HW constraint (probed): matmul tile_position must be diagonal (sp, sp) with
M <= 32, or plain (0, 0); mixed row/col strips wedge the device. So:
  - S1 (row gather) runs as diagonal strip matmuls, 4 instances per PSUM
    tile (one strip each);
  - levels 0-1 (M=100) run as (0,0) matmuls: a SBUF->SBUF DMA moves each
    group's gathered tables GT and column selections to partition strip 0
    (the level-0/1 column selections are computed full-width with
    strip = instance-group so one compare covers all 64 instances);
  - levels 2-4 (M <= 25 padded to 32) run as diagonal strip matmuls.
Row blocks pack 2-4 output rows per partition so DRAM writes are >= 400B
contiguous runs; output DMAs are batched over instance groups via 3-dim
access patterns (~60 DMA instructions per core total).
"""

import numpy as np

import concourse.bacc as bacc
import concourse.mybir as mybir
import concourse.tile as tile
from concourse.bass import AP
from concourse.bass_utils import run_bass_kernel_spmd

F32 = mybir.dt.float32

N_CORES = 8
PER_CORE = 64
N_QUADS = 16
N_GRP = 4  # quads per group -> 16 instances

G = 28
KP = 32
CTR = 13
INIT = 200
TOTAL = 53294

SIZES = [200, 100, 50, 25, 13]
RPP = [2, 1, 2, 1, 1]  # output rows per partition
MPART = [100, 100, 25, 25, 13]  # psum partitions (data rows per chunk)
DRAM_OFF = [0, 40000, 50000, 52500, 53125]

# gt column layout (per-instance gathered table, 428 cols):
#   L0: [0,200)   2 chunks of 100 (row = 2m+c)
#   L1: [200,300) identity (row = m)
#   L2: [300,364) 2 chunks of 32 (row = 2m+c, m<25, rest pad)
#   L3: [364,396) 1 chunk of 32 (m<25)
#   L4: [396,428) 1 chunk of 32 (m<13)
PW = [200, 100, 64, 32, 32]
RO = [0, 200, 300, 364, 396]
GT_W = 428
ST0_W = 300  # L0+L1 section moved to strip 0

# iota columns: R [0,428); C01 identity [428,728); C2-4 identity [728,816)
C01_RO = 428
C234_RO = [728, 778, 803]
IOTA_W = 816

# shifts columns
R_OFF = 0  # 5 levels x 16 quads (strip = quad-position g)
C01_OFF = 80  # 2 levels x 16 instances-in-group (strip = group)
C234_OFF = 112  # 3 levels x 16 quads (strip = quad-position g)
SH_W = 160


def _round_f32(x, scale):
    a = x.astype(np.float32) / np.float32(INIT)
    b = a * np.float32(scale)
    return np.round(b).astype(np.int32)


def _mk(t_ap, dims, extra_off=0):
    return AP(t_ap.tensor, t_ap.offset + extra_off, dims)


def build_program():
    nc = bacc.Bacc("TRN2", target_bir_lowering=False, debug=False)

    tbl_d = nc.dram_tensor("table4", [128, KP], F32, kind="ExternalInput")
    iota_d = nc.dram_tensor("iota", [128, IOTA_W], F32, kind="ExternalInput")
    shf_d = nc.dram_tensor("shifts", [128, SH_W], F32, kind="ExternalInput")
    out_d = nc.dram_tensor("out", [PER_CORE, TOTAL], F32, kind="ExternalOutput")

    def oap(n0, lvl, dims):
        return AP(out_d.ap().tensor, n0 * TOTAL + DRAM_OFF[lvl], dims)

    with tile.TileContext(nc) as tc:
        from contextlib import ExitStack

        with ExitStack() as ctx:
            consts = ctx.enter_context(tc.tile_pool(name="consts", bufs=1))
            selp = ctx.enter_context(tc.tile_pool(name="sel", bufs=1))
            c01p = ctx.enter_context(tc.tile_pool(name="c01", bufs=2))
            gtgp = ctx.enter_context(tc.tile_pool(name="gtg", bufs=2))
            gt0p = ctx.enter_context(tc.tile_pool(name="gt0", bufs=2))
            st0p = ctx.enter_context(tc.tile_pool(name="st0", bufs=3))
            stsp = ctx.enter_context(tc.tile_pool(name="stsm", bufs=2))
            psA = ctx.enter_context(tc.tile_pool(name="psA", bufs=2, space="PSUM"))
            psL0 = ctx.enter_context(tc.tile_pool(name="psL0", bufs=2, space="PSUM"))
            psSm = ctx.enter_context(tc.tile_pool(name="psSm", bufs=2, space="PSUM"))

            tbl = consts.tile([128, KP], F32, tag="tbl")
            nc.sync.dma_start(tbl[:, :], tbl_d[:, :])
            iot = consts.tile([128, IOTA_W], F32, tag="iota")
            nc.sync.dma_start(iot[:, :], iota_d[:, :])
            shf = consts.tile([128, SH_W], F32, tag="shf")
            nc.sync.dma_start(shf[:, :], shf_d[:, :])
            pi = iot[:, :].ap[0]
            ps_ = shf[:, :].ap[0]

            # R selections [128, 16*428], quad-major, strip = g
            selR = selp.tile([128, N_QUADS * GT_W], F32, tag="selR")
            prs = selR[:, :].ap[0]
            for lv in range(5):
                w = PW[lv]
                nc.vector.tensor_tensor(
                    _mk(selR[:, :], [prs, [GT_W, N_QUADS], [1, w]], RO[lv]),
                    _mk(iot[:, :], [pi, [0, N_QUADS], [1, w]], RO[lv]),
                    _mk(shf[:, :], [ps_, [1, N_QUADS], [0, w]], R_OFF + lv * 16),
                    mybir.AluOpType.is_equal,
                )

            # C selections levels 0-1 [128, 16*300], instance-major,
            # strip = GROUP (one compare per level covers all 64 instances)
            selC01 = selp.tile([128, 16 * ST0_W], F32, tag="selC01")
            pc = selC01[:, :].ap[0]
            for lv, off, w in ((0, 0, 200), (1, 200, 100)):
                nc.vector.tensor_tensor(
                    _mk(selC01[:, :], [pc, [ST0_W, 16], [1, w]], off),
                    _mk(iot[:, :], [pi, [0, 16], [1, w]], C01_RO + off),
                    _mk(shf[:, :], [ps_, [1, 16], [0, w]], C01_OFF + lv * 16),
                    mybir.AluOpType.is_equal,
                )

            # C selections levels 2-4, strip-packed (strip = g)
            selC = {}
            for i, lv in enumerate((2, 3, 4)):
                w = SIZES[lv]
                t = selp.tile([128, N_QUADS * w], F32, tag=f"selC{lv}", name=f"selC{lv}")
                nc.vector.tensor_tensor(
                    t[:, :],
                    _mk(iot[:, :], [pi, [0, N_QUADS], [1, w]], C234_RO[i]),
                    _mk(shf[:, :], [ps_, [1, N_QUADS], [0, w]], C234_OFF + i * 16),
                    mybir.AluOpType.is_equal,
                )
                selC[lv] = t

            lvcfg = [
                (2, 2, 50, "st2"),  # (lvl, chunks, N, tag)
                (3, 1, 25, "st3"),
                (4, 1, 13, "st4"),
            ]
            stg = {}

            for grp in range(N_GRP):
                sgp = 32 * grp

                # move this group's C01 selections to strip 0
                c01 = c01p.tile([32, 16 * ST0_W], F32, tag="c01")
                nc.scalar.dma_start(c01[:, :], selC01[sgp : sgp + 32, :])

                # S1 (diagonal) + gt copy into group tile
                gtg = gtgp.tile([128, N_GRP * GT_W], F32, tag="gtg")
                for qq in range(N_GRP):
                    q = grp * N_GRP + qq
                    ps1 = psA.tile([128, 512], F32, tag="s1")
                    for g in range(4):
                        sp = 32 * g
                        nc.tensor.matmul(
                            ps1[sp : sp + KP, 0:GT_W],
                            tbl[sp : sp + KP, :],
                            selR[sp : sp + KP, q * GT_W : (q + 1) * GT_W],
                            start=True,
                            stop=True,
                            tile_position=(sp, sp),
                        )
                    nc.scalar.copy(gtg[:, qq * GT_W : (qq + 1) * GT_W], ps1[:, 0:GT_W])

                # strip-move gt L0+L1 sections -> strip 0 [32, 16*300]
                gt0 = gt0p.tile([32, 16 * ST0_W], F32, tag="gt0")
                pg = gtg[:, :].ap[0]
                p0a = gt0[:, :].ap[0]
                for g in range(4):
                    sp = 32 * g
                    nc.sync.dma_start(
                        _mk(
                            gt0[:, :],
                            [[p0a[0], 32], [4 * ST0_W, N_GRP], [1, ST0_W]],
                            g * ST0_W,
                        ),
                        _mk(
                            gtg[sp : sp + 32, :],
                            [[pg[0], 32], [GT_W, N_GRP], [1, ST0_W]],
                        ),
                    )

                # S2 levels 0-1: (0,0) matmuls M=100 per instance
                for qq in range(N_GRP):
                    q = grp * N_GRP + qq
                    p1t = psL0.tile([100, 512], F32, tag="l1", name="l1")
                    for g in range(4):
                        i16 = qq * 4 + g
                        n = q * 4 + g
                        base = i16 * ST0_W
                        p0 = psL0.tile([100, 512], F32, tag="l0")
                        for c in range(2):
                            nc.tensor.matmul(
                                p0[:, c * 200 : (c + 1) * 200],
                                gt0[0:KP, base + c * 100 : base + (c + 1) * 100],
                                c01[0:KP, base : base + 200],
                                start=True,
                                stop=True,
                                tile_position=(0, 0),
                            )
                        nc.tensor.matmul(
                            p1t[:, g * 100 : (g + 1) * 100],
                            gt0[0:KP, base + 200 : base + 300],
                            c01[0:KP, base + 200 : base + 300],
                            start=True,
                            stop=True,
                            tile_position=(0, 0),
                        )
                        if i16 % 4 == 0:
                            st0 = st0p.tile([100, 4 * 400], F32, tag="st0")
                        (nc.vector.tensor_copy if g % 2 else nc.scalar.copy)(
                            st0[:, (i16 % 4) * 400 : (i16 % 4) * 400 + 400],
                            p0[:, 0:400],
                        )
                        if i16 % 4 == 3:
                            nc.sync.dma_start(
                                oap(n - 3, 0, [[400, 100], [TOTAL, 4], [1, 400]]),
                                st0[:, :],
                            )
                    if qq == 0:
                        st1 = st0p.tile([100, 16 * 100], F32, tag="st1", name="st1")
                    nc.vector.tensor_copy(
                        st1[:, qq * 400 : (qq + 1) * 400], p1t[:, 0:400]
                    )
                    if qq == N_GRP - 1:
                        nc.scalar.dma_start(
                            oap(grp * 16, 1, [[100, 100], [TOTAL, 16], [1, 100]]),
                            st1[:, :],
                        )

                # S2 levels 2-4: diagonal strip matmuls per quad
                if grp % 2 == 0:
                    for lv, nch, N, tag in lvcfg:
                        stg[lv] = stsp.tile(
                            [128, 8 * nch * N], F32, tag=tag, name=tag
                        )
                for qq in range(N_GRP):
                    q = grp * N_GRP + qq
                    q8 = (grp % 2) * 4 + qq
                    for lv, nch, N, tag in lvcfg:
                        F = nch * N
                        ps = psSm.tile([128, 512], F32, tag="sm")
                        for g in range(4):
                            sp = 32 * g
                            for c in range(nch):
                                nc.tensor.matmul(
                                    ps[sp : sp + 32, c * N : (c + 1) * N],
                                    gtg[
                                        sp : sp + KP,
                                        qq * GT_W + RO[lv] + c * 32 : qq * GT_W
                                        + RO[lv]
                                        + (c + 1) * 32,
                                    ],
                                    selC[lv][sp : sp + KP, q * N : (q + 1) * N],
                                    start=True,
                                    stop=True,
                                    tile_position=(sp, sp),
                                )
                        (nc.vector.tensor_copy if lv == 2 else nc.scalar.copy)(
                            stg[lv][:, q8 * F : (q8 + 1) * F], ps[:, 0:F]
                        )
                # small-level DMAs once per 2 groups: per strip, 8 quads
                if grp % 2 == 1:
                    n0g = (grp - 1) * 16
                    for lv, nch, N, tag in lvcfg:
                        F = nch * N
                        rows = MPART[lv]
                        pstg = stg[lv][:, :].ap[0]
                        for g in range(4):
                            sp = 32 * g
                            nc.scalar.dma_start(
                                oap(n0g + g, lv, [[F, rows], [4 * TOTAL, 8], [1, F]]),
                                _mk(
                                    stg[lv][sp : sp + rows, :],
                                    [[pstg[0], rows], [F, 8], [1, F]],
                                ),
                            )

    nc.compile()
    return nc


_PROGRAM = None


def _get_program():
    global _PROGRAM
    if _PROGRAM is None:
        _PROGRAM = build_program()
    return _PROGRAM


def host_inputs(pos, shared_mask):
    pos = np.asarray(pos)
    shared_mask = np.asarray(shared_mask, dtype=np.float32)

    table4 = np.zeros((128, KP), np.float32)
    iota = np.full((128, IOTA_W), -1e9, np.float32)
    for g in range(4):
        sp = 32 * g
        table4[sp : sp + G, :G] = shared_mask
        a = np.arange(G, dtype=np.float32)[:, None]
        for lv in range(5):
            r, M = RPP[lv], MPART[lv]
            pm = PW[lv] // r
            colvals = np.full(PW[lv], np.nan, np.float32)
            for c in range(r):
                for m in range(M):
                    colvals[c * pm + m] = r * m + c
            blk = colvals[None, :] - a
            blk = np.where(np.isnan(blk), -1e9, blk)
            iota[sp : sp + G, RO[lv] : RO[lv] + PW[lv]] = blk
        j = np.arange(200, dtype=np.float32)[None, :]
        iota[sp : sp + G, C01_RO : C01_RO + 200] = j - a
        iota[sp : sp + G, C01_RO + 200 : C01_RO + 300] = j[:, :100] - a
        for i, lv in enumerate((2, 3, 4)):
            S = SIZES[lv]
            iota[sp : sp + G, C234_RO[i] : C234_RO[i] + S] = j[:, :S] - a

    sh = np.stack([_round_f32(pos[:, 0], S) - CTR for S in SIZES], 0)
    sw = np.stack([_round_f32(pos[:, 1], S) - CTR for S in SIZES], 0)

    in_maps = []
    for c in range(N_CORES):
        shifts = np.zeros((128, SH_W), np.float32)
        for g in range(4):
            sp = 32 * g
            for q in range(N_QUADS):
                n = c * PER_CORE + q * 4 + g
                for lv in range(5):
                    shifts[sp : sp + 32, R_OFF + lv * 16 + q] = sh[lv, n]
                for i, lv in enumerate((2, 3, 4)):
                    shifts[sp : sp + 32, C234_OFF + i * 16 + q] = sw[lv, n]
        for grp in range(4):  # strip = group for C01
            sp = 32 * grp
            for i16 in range(16):
                n = c * PER_CORE + grp * 16 + i16
                shifts[sp : sp + 32, C01_OFF + i16] = sw[0, n]
                shifts[sp : sp + 32, C01_OFF + 16 + i16] = sw[1, n]
        in_maps.append(
            {"table4": table4, "iota": iota, "shifts": shifts.astype(np.float32)}
        )
    return in_maps


def kernel(pos, shared_mask):
    nc = _get_program()
    in_maps = host_inputs(pos, shared_mask)
    res = run_bass_kernel_spmd(nc, in_maps, list(range(N_CORES)))
    return np.concatenate([res.results[c]["out"] for c in range(N_CORES)], axis=0)
